# revision 2
# baseline (speedup 1.0000x reference)
"""Butterfly permuter kernel for Trainium2 (8 NeuronCores, SPMD data-parallel).

The reference applies 10 butterfly rotation stages along the feature axis
(dim=1024) of x [16384, 1024].  Each row is transformed independently, and the
10 stages compose into a single dense 1024x1024 orthogonal matrix R with
y_rows = x_rows @ R.  We compute R on the host in float64 from `angles`, then
run a tiled matmul on each core:

  per core: x_shard [2048, 1024]
  - DMA x in 2 MiB megatiles [128 part, 4096] (4 row-subtiles of 128 tokens)
  - PE-transpose each [128 tok, 128 dim] block (float32r, via identity) to get
    X^T blocks (contraction dim on partitions), evacuate PSUM->SBUF on ScalarE
  - 16 accumulating float32r matmuls per subtile: psum_y[jh] += XT_kb^T @ R_kb
    (float32r streams 1 cycle/row at N=512 - full PE rate, ~fp32 storage)
  - evacuate y PSUM->SBUF on VectorE, DMA out 2 MiB megatiles

Inputs arrive full-size; sharding is across the token axis (2048 rows/core).
"""

import numpy as np

import concourse.bass as bass
import concourse.mybir as mybir
import concourse.tile as tile
from concourse import bacc
from concourse.bass_utils import run_bass_kernel_spmd

N_CORES = 8
DIM = 1024
NUM_STAGES = 10
N_TOKENS = 16384
TOK_PER_CORE = N_TOKENS // N_CORES  # 2048
SUB = 128  # tokens per subtile (partition dim)
SUBTILES_PER_MEGA = 4
MEGA_ROWS = SUB * SUBTILES_PER_MEGA  # 512 tokens per DMA megatile
N_MEGA = TOK_PER_CORE // MEGA_ROWS  # 4
KB = DIM // 128  # 8 contraction blocks

F32 = mybir.dt.float32
F32R = mybir.dt.float32r


def compose_transform(angles: np.ndarray) -> np.ndarray:
    """Compose the 10 butterfly stages into R (float32) with y = x @ R."""
    y = np.eye(DIM, dtype=np.float64)
    a = np.asarray(angles, dtype=np.float64)
    for s in range(NUM_STAGES):
        span = 2 ** (s + 1)
        half = span // 2
        y = y.reshape(-1, DIM // span, span)
        left, right = y[..., :half], y[..., half:]
        th = a[s].reshape(1, DIM // span, half)
        c, sn = np.cos(th), np.sin(th)
        y = np.concatenate([c * left + sn * right, -sn * left + c * right], -1)
        y = y.reshape(-1, DIM)
    # row t of y is transform(e_t), so transform(x) = x @ y
    return np.ascontiguousarray(y, dtype=np.float32)


def build_bass(reps: int = 1):
    """reps>1 repeats the whole pipeline in one NEFF (for marginal timing)."""
    nc = bacc.Bacc(None, target_bir_lowering=False)
    x = nc.dram_tensor("x", [TOK_PER_CORE, DIM], F32, kind="ExternalInput")
    w = nc.dram_tensor("w", [DIM, DIM], F32, kind="ExternalInput")
    ident = nc.dram_tensor("ident", [128, 128], F32, kind="ExternalInput")
    y = nc.dram_tensor("y", [TOK_PER_CORE, DIM], F32, kind="ExternalOutput")

    n_sub = N_MEGA * SUBTILES_PER_MEGA  # 16 subtiles of 128 tokens

    # Variable-size DMA chunking (in units of 128-token subtiles): small
    # chunks at the start for a fast pipeline ramp, small at the end for a
    # short drain; 2-subtile (1 MiB) chunks in steady state.
    in_chunks = [1, 1, 2, 2, 2, 2, 2, 2, 2]
    out_chunks = [2, 2, 2, 2, 2, 2, 2, 1, 1]
    assert sum(in_chunks) == n_sub and sum(out_chunks) == n_sub
    in_start = [sum(in_chunks[:i]) for i in range(len(in_chunks))]
    out_start = [sum(out_chunks[:i]) for i in range(len(out_chunks))]
    sub_to_in_chunk = {}
    for ci, (st, ln) in enumerate(zip(in_start, in_chunks)):
        for s in range(st, st + ln):
            sub_to_in_chunk[s] = ci
    sub_to_out_chunk = {}
    for ci, (st, ln) in enumerate(zip(out_start, out_chunks)):
        for s in range(st, st + ln):
            sub_to_out_chunk[s] = ci

    with tile.TileContext(nc) as tc:
        with (
            tc.tile_pool(name="const", bufs=1) as const_pool,
            tc.tile_pool(name="wstage", bufs=3) as wstage_pool,
            tc.tile_pool(name="xin", bufs=3) as xin_pool,
            tc.tile_pool(name="xt", bufs=5) as xt_pool,
            tc.tile_pool(name="yout", bufs=3) as yout_pool,
            tc.tile_pool(name="pst", bufs=4, space="PSUM") as pst_pool,
            tc.tile_pool(name="psy", bufs=4, space="PSUM") as psy_pool,
        ):
            # identity goes via the SWDGE ring; the SP ring starts with the
            # first x chunk; W streams in behind it.
            ident_sb = const_pool.tile([128, 128], F32, name="ident_sb")
            nc.gpsimd.dma_start(ident_sb[:], ident[:])

            x_tiles = [None] * len(in_chunks)  # chunk idx -> (tile, start_sub)
            y_tiles = [None] * len(out_chunks)

            def load_chunk(ci):
                st, ln = in_start[ci], in_chunks[ci]
                x_tile = xin_pool.tile([128, ln * DIM], F32, name="x_chunk",
                                       tag="x_chunk",
                                       padded_shape=[128, 2 * DIM])
                r0 = st * SUB
                nc.sync.dma_start(
                    x_tile[:, : ln * DIM].rearrange("p (s c) -> p s c", c=DIM),
                    x[r0 : r0 + ln * SUB, :].rearrange("(s p) c -> p s c", p=128),
                )
                x_tiles[ci] = x_tile

            load_chunk(0)
            first_load_done = True

            # W: DMA [jh][kb] blocks of [128,512] (j-half-major so the first
            # 2 MiB unblocks the first matmul group) on the ACT HWDGE ring,
            # then round fp32 -> f32r on DVE (walrus requires f32r matmul
            # inputs to come from a rounding instruction).
            w_sbr = const_pool.tile([128, KB * DIM], F32R, name="w_sbr")

            def w_off(jh, kb):
                return (jh * KB + kb) * 512

            for jh in range(2):
                for kb in range(KB):
                    w_stage = wstage_pool.tile([128, 512], F32, name="w_stage",
                                               tag="w_stage")
                    nc.sync.dma_start(
                        w_stage[:],
                        w[kb * 128 : (kb + 1) * 128, jh * 512 : (jh + 1) * 512],
                    )
                    off = w_off(jh, kb)
                    nc.vector.tensor_copy(w_sbr[:, off : off + 512], w_stage[:])

            xts = [None] * n_sub

            def emit_transpose(s):
                ci = sub_to_in_chunk[s]
                xcol = (s - in_start[ci]) * DIM
                x_tile = x_tiles[ci]
                ps_t0 = pst_pool.tile([128, 512], F32, name="ps_t0", tag="ps_t")
                ps_t1 = pst_pool.tile([128, 512], F32, name="ps_t1", tag="ps_t")
                for kb in range(KB):
                    dst = ps_t0 if kb < 4 else ps_t1
                    j = (kb % 4) * 128
                    nc.tensor.transpose(
                        dst[:, j : j + 128],
                        x_tile[:, xcol + kb * 128 : xcol + (kb + 1) * 128],
                        ident_sb,
                    )
                xt = xt_pool.tile([128, DIM], F32R, name="xt", tag="xt")
                nc.scalar.copy(xt[:, :512], ps_t0[:])
                nc.scalar.copy(xt[:, 512:], ps_t1[:])
                xts[s] = xt

            def emit_matmul(s, jh):
                co = sub_to_out_chunk[s]
                st, ln = out_start[co], out_chunks[co]
                if s == st and jh == 0:
                    y_tiles[co] = yout_pool.tile(
                        [128, ln * DIM], F32, name="y_chunk", tag="y_chunk",
                        padded_shape=[128, 2 * DIM],
                    )
                y_tile = y_tiles[co]
                ycol = (s - st) * DIM + jh * 512
                xt = xts[s]
                ps_y = psy_pool.tile([128, 512], F32, name="ps_y", tag="ps_y")
                for kb in range(KB):
                    off = (jh * KB + kb) * 512
                    nc.tensor.matmul(
                        ps_y[:],
                        xt[:, kb * 128 : (kb + 1) * 128],
                        w_sbr[:, off : off + 512],
                        start=(kb == 0),
                        stop=(kb == KB - 1),
                    )
                nc.vector.tensor_copy(y_tile[:, ycol : ycol + 512], ps_y[:])
                if s == st + ln - 1 and jh == 1:
                    r0 = st * SUB
                    # y stores go out on the ACT HWDGE ring so they don't
                    # queue ahead of later x loads on the SP ring.
                    nc.scalar.dma_start(
                        y[r0 : r0 + ln * SUB, :].rearrange("(s p) c -> p s c", p=128),
                        y_tile[:, : ln * DIM].rearrange("p (s c) -> p s c", c=DIM),
                    )

            # Skewed software pipeline: transposes run one subtile ahead of
            # the matmuls so the PE never waits on the ScalarE PSUM->SBUF
            # evacuation of its own transpose outputs.
            # Transposes run two subtiles ahead of the matmuls (more PE
            # runway while W streams in), and j-halves are staggered one
            # subtile apart: MM(s, jh0) then MM(s-1, jh1), so subtile 0's
            # jh1 group (which needs the second half of W) doesn't stall
            # the in-order PE stream at startup.
            SKEW = 2
            for _rep in range(reps):
                if not first_load_done:
                    load_chunk(0)
                first_load_done = False
                for p in range(min(SKEW, n_sub)):
                    ci = sub_to_in_chunk[p]
                    if p == in_start[ci] and p > 0:
                        load_chunk(ci)
                    emit_transpose(p)
                for s in range(n_sub):
                    nxt = s + SKEW
                    if nxt < n_sub:
                        ci = sub_to_in_chunk[nxt]
                        if nxt == in_start[ci]:
                            load_chunk(ci)
                        emit_transpose(nxt)
                    emit_matmul(s, 0)
                    if s >= 1:
                        emit_matmul(s - 1, 1)
                emit_matmul(n_sub - 1, 1)
    nc.compile()
    return nc


_NC_CACHE = None


def _get_nc():
    global _NC_CACHE
    if _NC_CACHE is None:
        _NC_CACHE = build_bass()
    return _NC_CACHE


def make_core_inputs(x: np.ndarray, angles: np.ndarray) -> list[dict]:
    """Per-core input maps (shared by run() and bench.py)."""
    x = np.ascontiguousarray(np.asarray(x, dtype=np.float32))
    w = compose_transform(angles)
    ident = np.eye(128, dtype=np.float32)
    return [
        {
            "x": x[c * TOK_PER_CORE : (c + 1) * TOK_PER_CORE],
            "w": w,
            "ident": ident,
        }
        for c in range(N_CORES)
    ]


def run(x: np.ndarray, angles: np.ndarray, trace: bool = False):
    """Run on 8 cores; returns (y_full, BassKernelResults)."""
    nc = _get_nc()
    in_maps = make_core_inputs(x, angles)
    res = run_bass_kernel_spmd(
        nc, in_maps, core_ids=list(range(N_CORES)), trace=trace
    )
    y = np.concatenate([res.results[c]["y"] for c in range(N_CORES)], axis=0)
    return y, res


def kernel(x: np.ndarray, angles: np.ndarray) -> np.ndarray:
    y, _ = run(x, angles, trace=False)
    return y



# revision 4
# speedup vs baseline: 1.3778x; 1.3778x over previous
"""Butterfly permuter kernel for Trainium2 (8 NeuronCores, SPMD data-parallel).

The reference applies 10 butterfly rotation stages along the feature axis
(dim=1024) of x [16384, 1024].  Each row is transformed independently, and the
10 stages compose into a single dense 1024x1024 orthogonal matrix R with
y_rows = x_rows @ R.  We compute R on the host in float64 from `angles`, then
run a tiled matmul on each core (bf16 inputs, fp32 PSUM accumulate):

  per core: x_shard [2048, 1024]
  - DMA x in 2 MiB megatiles [128 part, 4096] (4 row-subtiles of 128 tokens)
  - round fp32 -> bf16 on DVE (2x_2p single-src mode)
  - PE-transpose each [128 tok, 128 dim] bf16 block (1 cyc/row) to get
    X^T blocks (contraction dim on partitions), evacuate PSUM->SBUF on ScalarE
    casting to bf16
  - 16 accumulating bf16 matmuls per subtile: psum_y[jh] += XT_kb^T @ R_kb
    (bf16 streams 1 cycle/row - full PE rate; W is shipped as bf16 from host)
  - evacuate y PSUM->SBUF on VectorE (fp32), DMA out 2 MiB megatiles

Inputs arrive full-size; sharding is across the token axis (2048 rows/core).
"""

import numpy as np

import concourse.bass as bass
import concourse.mybir as mybir
import concourse.tile as tile
from concourse import bacc
from concourse.bass_utils import run_bass_kernel_spmd

N_CORES = 8
DIM = 1024
NUM_STAGES = 10
N_TOKENS = 16384
TOK_PER_CORE = N_TOKENS // N_CORES  # 2048
SUB = 128  # tokens per subtile (partition dim)
SUBTILES_PER_MEGA = 4
MEGA_ROWS = SUB * SUBTILES_PER_MEGA  # 512 tokens per DMA megatile
N_MEGA = TOK_PER_CORE // MEGA_ROWS  # 4
KB = DIM // 128  # 8 contraction blocks

F32 = mybir.dt.float32
BF16 = mybir.dt.bfloat16


def compose_transform(angles: np.ndarray) -> np.ndarray:
    """Compose the 10 butterfly stages into R (float64) with y = x @ R."""
    y = np.eye(DIM, dtype=np.float64)
    a = np.asarray(angles, dtype=np.float64)
    for s in range(NUM_STAGES):
        span = 2 ** (s + 1)
        half = span // 2
        y = y.reshape(-1, DIM // span, span)
        left, right = y[..., :half], y[..., half:]
        th = a[s].reshape(1, DIM // span, half)
        c, sn = np.cos(th), np.sin(th)
        y = np.concatenate([c * left + sn * right, -sn * left + c * right], -1)
        y = y.reshape(-1, DIM)
    # row t of y is transform(e_t), so transform(x) = x @ y
    return y


def build_bass(reps: int = 1):
    """reps>1 repeats the whole pipeline in one NEFF (for marginal timing)."""
    nc = bacc.Bacc(None, target_bir_lowering=False)
    x = nc.dram_tensor("x", [TOK_PER_CORE, DIM], F32, kind="ExternalInput")
    w = nc.dram_tensor("w", [DIM, DIM], BF16, kind="ExternalInput")
    ident = nc.dram_tensor("ident", [128, 128], BF16, kind="ExternalInput")
    y = nc.dram_tensor("y", [TOK_PER_CORE, DIM], F32, kind="ExternalOutput")

    n_sub = N_MEGA * SUBTILES_PER_MEGA  # 16 subtiles of 128 tokens

    # Variable-size DMA chunking (in units of 128-token subtiles): small
    # chunks at the start for a fast pipeline ramp, small at the end for a
    # short drain; 2-subtile (1 MiB) chunks in steady state.
    in_chunks = [1, 1, 2, 2, 2, 2, 2, 2, 2]
    out_chunks = [2, 2, 2, 2, 2, 2, 2, 1, 1]
    assert sum(in_chunks) == n_sub and sum(out_chunks) == n_sub
    in_start = [sum(in_chunks[:i]) for i in range(len(in_chunks))]
    out_start = [sum(out_chunks[:i]) for i in range(len(out_chunks))]
    sub_to_in_chunk = {}
    for ci, (st, ln) in enumerate(zip(in_start, in_chunks)):
        for s in range(st, st + ln):
            sub_to_in_chunk[s] = ci
    sub_to_out_chunk = {}
    for ci, (st, ln) in enumerate(zip(out_start, out_chunks)):
        for s in range(st, st + ln):
            sub_to_out_chunk[s] = ci

    with tile.TileContext(nc) as tc:
        with (
            tc.tile_pool(name="const", bufs=1) as const_pool,
            tc.tile_pool(name="xin", bufs=3) as xin_pool,
            tc.tile_pool(name="xbf", bufs=3) as xbf_pool,
            tc.tile_pool(name="xt", bufs=5) as xt_pool,
            tc.tile_pool(name="yout", bufs=3) as yout_pool,
            tc.tile_pool(name="pst", bufs=4, space="PSUM") as pst_pool,
            tc.tile_pool(name="psy", bufs=4, space="PSUM") as psy_pool,
        ):
            # identity goes via the SWDGE ring; the SP ring starts with the
            # first x chunk; W streams in behind it.
            ident_sb = const_pool.tile([128, 128], BF16, name="ident_sb")
            nc.gpsimd.dma_start(ident_sb[:], ident[:])

            x_tiles = [None] * len(in_chunks)  # chunk idx -> bf16 tile
            y_tiles = [None] * len(out_chunks)

            def load_chunk(ci):
                st, ln = in_start[ci], in_chunks[ci]
                x_tile = xin_pool.tile([128, ln * DIM], F32, name="x_chunk",
                                       tag="x_chunk",
                                       padded_shape=[128, 2 * DIM])
                r0 = st * SUB
                nc.sync.dma_start(
                    x_tile[:, : ln * DIM].rearrange("p (s c) -> p s c", c=DIM),
                    x[r0 : r0 + ln * SUB, :].rearrange("(s p) c -> p s c", p=128),
                )
                # fp32 -> bf16 round on DVE (single-src SBUF->SBUF: 2x_2p)
                xb_tile = xbf_pool.tile([128, ln * DIM], BF16, name="xb_chunk",
                                        tag="xb_chunk",
                                        padded_shape=[128, 2 * DIM])
                nc.vector.tensor_copy(xb_tile[:, : ln * DIM],
                                      x_tile[:, : ln * DIM])
                x_tiles[ci] = xb_tile

            load_chunk(0)
            first_load_done = True

            # W: bf16 from host; DMA [jh][kb] blocks of [128,512] directly
            # into the resident SBUF bank (j-half-major so the first 1 MiB
            # unblocks the first matmul group) on the ACT HWDGE ring.
            w_sbr = const_pool.tile([128, KB * DIM], BF16, name="w_sbr")

            def w_off(jh, kb):
                return (jh * KB + kb) * 512

            for jh in range(2):
                for kb in range(KB):
                    off = w_off(jh, kb)
                    nc.scalar.dma_start(
                        w_sbr[:, off : off + 512],
                        w[kb * 128 : (kb + 1) * 128, jh * 512 : (jh + 1) * 512],
                    )

            xts = [None] * n_sub

            def emit_transpose(s):
                ci = sub_to_in_chunk[s]
                xcol = (s - in_start[ci]) * DIM
                x_tile = x_tiles[ci]
                ps_t0 = pst_pool.tile([128, 512], BF16, name="ps_t0", tag="ps_t")
                ps_t1 = pst_pool.tile([128, 512], BF16, name="ps_t1", tag="ps_t")
                for kb in range(KB):
                    dst = ps_t0 if kb < 4 else ps_t1
                    j = (kb % 4) * 128
                    nc.tensor.transpose(
                        dst[:, j : j + 128],
                        x_tile[:, xcol + kb * 128 : xcol + (kb + 1) * 128],
                        ident_sb,
                    )
                xt = xt_pool.tile([128, DIM], BF16, name="xt", tag="xt")
                nc.scalar.copy(xt[:, :512], ps_t0[:])
                nc.scalar.copy(xt[:, 512:], ps_t1[:])
                xts[s] = xt

            def emit_matmul(s, jh):
                co = sub_to_out_chunk[s]
                st, ln = out_start[co], out_chunks[co]
                if s == st and jh == 0:
                    y_tiles[co] = yout_pool.tile(
                        [128, ln * DIM], F32, name="y_chunk", tag="y_chunk",
                        padded_shape=[128, 2 * DIM],
                    )
                y_tile = y_tiles[co]
                ycol = (s - st) * DIM + jh * 512
                xt = xts[s]
                ps_y = psy_pool.tile([128, 512], F32, name="ps_y", tag="ps_y")
                for kb in range(KB):
                    off = (jh * KB + kb) * 512
                    nc.tensor.matmul(
                        ps_y[:],
                        xt[:, kb * 128 : (kb + 1) * 128],
                        w_sbr[:, off : off + 512],
                        start=(kb == 0),
                        stop=(kb == KB - 1),
                    )
                nc.vector.tensor_copy(y_tile[:, ycol : ycol + 512], ps_y[:])
                if s == st + ln - 1 and jh == 1:
                    r0 = st * SUB
                    # y stores go out on the ACT HWDGE ring so they don't
                    # queue ahead of later x loads on the SP ring.
                    nc.scalar.dma_start(
                        y[r0 : r0 + ln * SUB, :].rearrange("(s p) c -> p s c", p=128),
                        y_tile[:, : ln * DIM].rearrange("p (s c) -> p s c", c=DIM),
                    )

            # Skewed software pipeline: transposes run two subtiles ahead of
            # the matmuls (PE runway while W streams in), and j-halves are
            # staggered one subtile apart: MM(s, jh0) then MM(s-1, jh1).
            SKEW = 2
            for _rep in range(reps):
                if not first_load_done:
                    load_chunk(0)
                first_load_done = False
                for p in range(min(SKEW, n_sub)):
                    ci = sub_to_in_chunk[p]
                    if p == in_start[ci] and p > 0:
                        load_chunk(ci)
                    emit_transpose(p)
                for s in range(n_sub):
                    nxt = s + SKEW
                    if nxt < n_sub:
                        ci = sub_to_in_chunk[nxt]
                        if nxt == in_start[ci]:
                            load_chunk(ci)
                        emit_transpose(nxt)
                    emit_matmul(s, 0)
                    if s >= 1:
                        emit_matmul(s - 1, 1)
                emit_matmul(n_sub - 1, 1)
    nc.compile()
    return nc


_NC_CACHE = None


def _get_nc():
    global _NC_CACHE
    if _NC_CACHE is None:
        _NC_CACHE = build_bass()
    return _NC_CACHE


def make_core_inputs(x: np.ndarray, angles: np.ndarray) -> list[dict]:
    """Per-core input maps (shared by run() and bench.py)."""
    import ml_dtypes

    x = np.ascontiguousarray(np.asarray(x, dtype=np.float32))
    w = compose_transform(angles).astype(ml_dtypes.bfloat16)
    ident = np.eye(128, dtype=ml_dtypes.bfloat16)
    return [
        {
            "x": x[c * TOK_PER_CORE : (c + 1) * TOK_PER_CORE],
            "w": w,
            "ident": ident,
        }
        for c in range(N_CORES)
    ]


def run(x: np.ndarray, angles: np.ndarray, trace: bool = False):
    """Run on 8 cores; returns (y_full, BassKernelResults)."""
    nc = _get_nc()
    in_maps = make_core_inputs(x, angles)
    res = run_bass_kernel_spmd(
        nc, in_maps, core_ids=list(range(N_CORES)), trace=trace
    )
    y = np.concatenate([res.results[c]["y"] for c in range(N_CORES)], axis=0)
    return y, res


def kernel(x: np.ndarray, angles: np.ndarray) -> np.ndarray:
    y, _ = run(x, angles, trace=False)
    return y


# revision 15
# speedup vs baseline: 1.6640x; 1.2077x over previous
"""Butterfly permuter kernel for Trainium2 (8 NeuronCores, SPMD data-parallel).

The reference applies 10 butterfly rotation stages along the feature axis
(dim=1024) of x [16384, 1024].  Stages 0..8 (spans 2..512) compose into a
block-diagonal matrix A8 with two 512x512 blocks, and stage 9 (span 1024)
is an elementwise rotation pairing column c with c+512.  So instead of one
dense 1024x1024 matmul (PE cost 131k cycles/core) we do:

  y = stage9( x @ A8 )          with A8 block-diagonal (PE cost 65k cycles)

Per core (2048 tokens), per rep:
  - gpsimd (SWDGE) casting DMA: x fp32 DRAM -> bf16 SBUF megatiles
  - PE-transpose each [128 tok, 128 dim] bf16 block (1 cyc/row) -> X^T
    blocks with the contraction dim on partitions; ScalarE evacuates
    PSUM->SBUF into per-group [128, kb-major x 256 tok] bf16 tiles
  - BD matmul: for each 128-col output block j, 4 accumulating bf16
    matmuls (K=128 each) -> PSUM [128 col, 256 tok] fp32 = Y0^T blocks
    (transposed layout); ScalarE evacuates into 8 SBUF tiles
    yt_j [128 col, 2048 tok] bf16
  - stage 9 on VectorE in transposed layout: cols are partitions, so the
    per-column cos/sin are per-partition scalars: 4 tensor_scalar (4x mode)
    + 2 tensor_tensor (2x mode) ops per block pair (j, j+4), bf16
  - PE-transpose back to [tok, col] (bf16, 1 cyc/row), GpSimd evacuates
    PSUM->SBUF as fp32, HWDGE DMA out

Work is software-pipelined across reps: stage 9 of rep r and the
back-transposes of rep r-1 execute during rep r+1's front end.
"""

import numpy as np

import concourse.bass as bass
import concourse.mybir as mybir
import concourse.tile as tile
from concourse import bacc
from concourse.bass_utils import run_bass_kernel_spmd

N_CORES = 8
DIM = 1024
NUM_STAGES = 10
N_TOKENS = 16384
TOK_PER_CORE = N_TOKENS // N_CORES  # 2048
SUB = 128  # tokens per subtile (partition dim)
N_SUB = TOK_PER_CORE // SUB  # 16 subtiles
GRP = 2  # subtiles per matmul group (256-token moving operand)
N_GRP = N_SUB // GRP  # 8 groups
IN_CHUNK = 4  # subtiles per input DMA (SWDGE cast DMA, 2 MiB DRAM-side)
N_IN = N_SUB // IN_CHUNK  # 4
OUT_CHUNK = 2  # subtiles per output DMA (1 MiB)
N_OUT = N_SUB // OUT_CHUNK  # 8

F32 = mybir.dt.float32
BF16 = mybir.dt.bfloat16
MULT = mybir.AluOpType.mult
ADD = mybir.AluOpType.add
SUBTRACT = mybir.AluOpType.subtract


def compose_transform(angles: np.ndarray, n_stages: int = NUM_STAGES) -> np.ndarray:
    """Compose the first n_stages butterfly stages (float64), y = x @ R."""
    y = np.eye(DIM, dtype=np.float64)
    a = np.asarray(angles, dtype=np.float64)
    for s in range(n_stages):
        span = 2 ** (s + 1)
        half = span // 2
        y = y.reshape(-1, DIM // span, span)
        left, right = y[..., :half], y[..., half:]
        th = a[s].reshape(1, DIM // span, half)
        c, sn = np.cos(th), np.sin(th)
        y = np.concatenate([c * left + sn * right, -sn * left + c * right], -1)
        y = y.reshape(-1, DIM)
    return y


def build_bass(reps: int = 1):
    """reps>1 repeats the whole pipeline in one NEFF (for marginal timing)."""
    nc = bacc.Bacc(None, target_bir_lowering=False)
    x = nc.dram_tensor("x", [TOK_PER_CORE, DIM], F32, kind="ExternalInput")
    # w: 32 stationary blocks [128 k-dims, 128 cols], slot j*4+k
    w = nc.dram_tensor("w", [128, 32 * 128], BF16, kind="ExternalInput")
    ident = nc.dram_tensor("ident", [128, 128], BF16, kind="ExternalInput")
    # coef: [:, 2j] = cos(theta9[j*128:(j+1)*128]), [:, 2j+1] = sin(...)
    coef = nc.dram_tensor("coef", [128, 8], F32, kind="ExternalInput")
    y = nc.dram_tensor("y", [TOK_PER_CORE, DIM], F32, kind="ExternalOutput")

    with tile.TileContext(nc) as tc:
        with (
            tc.tile_pool(name="const", bufs=1) as const_pool,
            tc.tile_pool(name="xbf", bufs=3) as xbf_pool,
            tc.tile_pool(name="xt", bufs=3) as xt_pool,
            tc.tile_pool(name="yt", bufs=2) as yt_pool,
            tc.tile_pool(name="tmp", bufs=8) as tmp_pool,
            tc.tile_pool(name="yout", bufs=3) as yout_pool,
            tc.tile_pool(name="pst", bufs=3, space="PSUM") as pst_pool,
            tc.tile_pool(name="psy", bufs=3, space="PSUM") as psy_pool,
            tc.tile_pool(name="psf", bufs=2, space="PSUM") as psf_pool,
        ):
            ident_sb = const_pool.tile([128, 128], BF16, name="ident_sb")
            nc.sync.dma_start(ident_sb[:], ident[:])
            coef_sb = const_pool.tile([128, 8], F32, name="coef_sb")
            nc.sync.dma_start(coef_sb[:], coef[:])
            w_sb = const_pool.tile([128, 32 * 128], BF16, name="w_sb")
            # j-major so the first out-blocks' weights arrive first
            for j in range(8):
                nc.scalar.dma_start(
                    w_sb[:, j * 512 : (j + 1) * 512],
                    w[:, j * 512 : (j + 1) * 512],
                )

            def cs_ap(j):
                return (
                    coef_sb[:, 2 * j : 2 * j + 1],
                    coef_sb[:, 2 * j + 1 : 2 * j + 2],
                )

            # ---- per-rep emission helpers ------------------------------
            def load_chunk(ci):
                """SWDGE casting DMA: 4 subtiles of x fp32 -> bf16 SBUF."""
                xb = xbf_pool.tile([128, IN_CHUNK * DIM], BF16, name="xb",
                                   tag="xb")
                r0 = ci * IN_CHUNK * SUB
                nc.gpsimd.dma_start(
                    xb[:].rearrange("p (s c) -> p s c", c=DIM),
                    x[r0 : r0 + IN_CHUNK * SUB, :].rearrange(
                        "(s p) c -> p s c", p=128
                    ),
                )
                return xb

            def emit_transpose(s, xb, xt_g):
                """Transpose subtile s's 8 blocks; evac into xt_g (kb-major).

                xt_g free layout: kb(8) x half(2) x tok(128); this subtile
                fills half h = s % GRP.
                """
                xcol = (s % IN_CHUNK) * DIM
                h = s % GRP
                # one PSUM bank holds all 8 transposed blocks of the subtile
                ps_t = pst_pool.tile([128, 1024], BF16, name="ps_t", tag="ps_t")
                for kb in range(8):
                    nc.tensor.transpose(
                        ps_t[:, kb * 128 : (kb + 1) * 128],
                        xb[:, xcol + kb * 128 : xcol + (kb + 1) * 128],
                        ident_sb,
                    )
                xtv = xt_g[:].rearrange("p (kb half t) -> p half kb t",
                                        half=GRP, t=128)
                # bf16 PSUM->SBUF on DVE runs in 2x_1p mode (658 ns/subtile),
                # cheaper than ScalarE and keeps ACT off the
                # transpose->matmul critical chain.
                nc.vector.tensor_copy(xtv[:, h : h + 1, :, :], ps_t[:])

            def emit_group_mm(g, xt_g, yts):
                """8 output blocks x 4 accumulating K=128 matmuls (bf16)."""
                for jp in range(4):  # two output blocks share one PSUM bank
                    ps_y = psy_pool.tile([128, 512], F32, name="ps_y", tag="ps_y")
                    for jh in range(2):
                        j = jp * 2 + jh
                        base_k = 0 if j < 4 else 4
                        for k in range(4):
                            kb = base_k + k
                            nc.tensor.matmul(
                                ps_y[:, jh * 256 : (jh + 1) * 256],
                                w_sb[:, (j * 4 + k) * 128 : (j * 4 + k + 1) * 128],
                                xt_g[:, kb * 256 : (kb + 1) * 256],
                                start=(k == 0),
                                stop=(k == 3),
                            )
                        nc.scalar.copy(
                            yts[j][:, g * 256 : (g + 1) * 256],
                            ps_y[:, jh * 256 : (jh + 1) * 256],
                        )

            def emit_stage9_pair(yts, j):
                """Transposed-layout rotation for pair (j, j+4): per-partition
                cos/sin scalars, bf16 (tensor_scalar 4x, tensor_tensor 2x)."""
                c_ap, s_ap = cs_ap(j)
                l, r = yts[j], yts[j + 4]
                t1 = tmp_pool.tile([128, TOK_PER_CORE], BF16, name="t1",
                                   tag="t9")
                t2 = tmp_pool.tile([128, TOK_PER_CORE], BF16, name="t2",
                                   tag="t9")
                t3 = tmp_pool.tile([128, TOK_PER_CORE], BF16, name="t3",
                                   tag="t9")
                t4 = tmp_pool.tile([128, TOK_PER_CORE], BF16, name="t4",
                                   tag="t9")
                nc.vector.tensor_scalar(t1[:], l[:], c_ap, None, MULT)
                nc.vector.tensor_scalar(t3[:], l[:], s_ap, None, MULT)
                nc.vector.tensor_scalar(t2[:], r[:], s_ap, None, MULT)
                nc.vector.tensor_scalar(t4[:], r[:], c_ap, None, MULT)
                nc.vector.tensor_tensor(l[:], t1[:], t2[:], ADD)
                nc.vector.tensor_tensor(r[:], t4[:], t3[:], SUBTRACT)

            def emit_back(rep_yts):
                """Back-transpose + fp32 evac (GpSimd) + output DMA."""
                y_sb = None
                for s in range(N_SUB):
                    if s % OUT_CHUNK == 0:
                        y_sb = yout_pool.tile(
                            [128, OUT_CHUNK * DIM], F32, name="y_sb", tag="y_sb"
                        )
                    base = (s % OUT_CHUNK) * DIM
                    ps_f0 = psf_pool.tile([128, 512], BF16, name="ps_f0",
                                          tag="ps_f")
                    ps_f1 = psf_pool.tile([128, 512], BF16, name="ps_f1",
                                          tag="ps_f")
                    for j in range(8):
                        dst = ps_f0 if j < 4 else ps_f1
                        jcol = (j % 4) * 128
                        nc.tensor.transpose(
                            dst[:, jcol : jcol + 128],
                            rep_yts[j][:, s * 128 : (s + 1) * 128],
                            ident_sb,
                        )
                    # alternate the two fp32 evacs between DVE and ScalarE so
                    # the psf-bank handoff is paced by neither alone (GpSimd
                    # cannot access PSUM on TRN2)
                    nc.vector.tensor_copy(y_sb[:, base : base + 512], ps_f0[:])
                    nc.scalar.copy(y_sb[:, base + 512 : base + DIM], ps_f1[:])
                    if s % OUT_CHUNK == OUT_CHUNK - 1:
                        r0 = (s - OUT_CHUNK + 1) * SUB
                        nc.scalar.dma_start(
                            y[r0 : r0 + OUT_CHUNK * SUB, :].rearrange(
                                "(s p) c -> p s c", p=128
                            ),
                            y_sb[:].rearrange("p (s c) -> p s c", c=DIM),
                        )

            # ---- software pipeline across reps -------------------------
            # Rep r's front end (loads/transposes/matmuls) interleaves the
            # stage-9 pairs of rep r-1 on DVE (their inputs are long since
            # ready, so DVE never convoys the transpose->matmul chain), then
            # emits rep r-1's back end (back-transposes + evac + store).
            prev_yts = None
            for _rep in range(reps):
                yts = [
                    yt_pool.tile([128, TOK_PER_CORE], BF16, name=f"yt{j}",
                                 tag=f"yt{j}")
                    for j in range(8)
                ]
                # transposes run one group ahead of the matmuls so the PE
                # never waits on the DVE PSUM->SBUF evacuation of its own
                # transpose outputs
                xb = load_chunk(0)
                xt_tiles = [None] * N_GRP

                def emit_group_tr(g, xb):
                    xt_g = xt_pool.tile([128, 8 * GRP * 128], BF16,
                                        name="xt_g", tag="xt_g")
                    for si in range(GRP):
                        emit_transpose(g * GRP + si, xb, xt_g)
                    xt_tiles[g] = xt_g

                emit_group_tr(0, xb)
                for g in range(N_GRP):
                    nxt = g + 1
                    if nxt < N_GRP:
                        if nxt * GRP % IN_CHUNK == 0:
                            ci = nxt * GRP // IN_CHUNK
                            if ci < N_IN:
                                xb = load_chunk(ci)
                        emit_group_tr(nxt, xb)
                    emit_group_mm(g, xt_tiles[g], yts)
                    if g % 2 == 1 and prev_yts is not None:
                        emit_stage9_pair(prev_yts, g // 2)
                if prev_yts is not None:
                    emit_back(prev_yts)
                prev_yts = yts
            for j in range(4):
                emit_stage9_pair(prev_yts, j)
            emit_back(prev_yts)
    nc.compile()
    return nc


_NC_CACHE = None


def _get_nc():
    global _NC_CACHE
    if _NC_CACHE is None:
        _NC_CACHE = build_bass()
    return _NC_CACHE


def make_core_inputs(x: np.ndarray, angles: np.ndarray) -> list[dict]:
    """Per-core input maps (shared by run() and bench.py)."""
    import ml_dtypes

    x = np.ascontiguousarray(np.asarray(x, dtype=np.float32))
    angles = np.asarray(angles, dtype=np.float64)
    A8 = compose_transform(angles, 9)
    w = np.empty((128, 32 * 128), dtype=np.float64)
    for j in range(8):
        base_k = 0 if j < 4 else 4
        for k in range(4):
            blk = A8[(base_k + k) * 128 : (base_k + k + 1) * 128,
                     j * 128 : (j + 1) * 128]
            w[:, (j * 4 + k) * 128 : (j * 4 + k + 1) * 128] = blk
    w = w.astype(ml_dtypes.bfloat16)
    th9 = angles[9]
    coef = np.empty((128, 8), dtype=np.float32)
    for j in range(4):
        coef[:, 2 * j] = np.cos(th9[j * 128 : (j + 1) * 128])
        coef[:, 2 * j + 1] = np.sin(th9[j * 128 : (j + 1) * 128])
    ident = np.eye(128, dtype=ml_dtypes.bfloat16)
    return [
        {
            "x": x[c * TOK_PER_CORE : (c + 1) * TOK_PER_CORE],
            "w": w,
            "ident": ident,
            "coef": coef,
        }
        for c in range(N_CORES)
    ]


def run(x: np.ndarray, angles: np.ndarray, trace: bool = False):
    """Run on 8 cores; returns (y_full, BassKernelResults)."""
    nc = _get_nc()
    in_maps = make_core_inputs(x, angles)
    res = run_bass_kernel_spmd(
        nc, in_maps, core_ids=list(range(N_CORES)), trace=trace
    )
    y = np.concatenate([res.results[c]["y"] for c in range(N_CORES)], axis=0)
    return y, res


def kernel(x: np.ndarray, angles: np.ndarray) -> np.ndarray:
    y, _ = run(x, angles, trace=False)
    return y
